# revision 1
# baseline (speedup 1.0000x reference)
"""Trainium2 Bass kernel for nn_Discriminator (W_down projection + time-embed
+ W_vt/W_ih projections + 16-step LSTM + linear head).

Strategy: phase A (the dominant 42-GFLOP W_down contraction) is sharded over
the CONTRACTION dim N across the 8 cores: core k holds W_down columns
[k*2500, (k+1)*2500) (2.6 MB fp16 instead of 20.6 MB replicated) and the
matching slice of v for ALL batches. Each core computes partial
vdT [512, 2048] for the full batch; fp16 ReduceScatters (CCE adds on the
SDMA path, compute engines stay free) hand core j the reduced vdT [512, 256]
for its batch block. This turns phase A from DMA-bound (43 MB/core) into
compute-bound (~68 us).

The reduction is split into two collectives by l-half (columns l<8 / l>=8 of
every batch block): phase A computes the l<8 columns first, so RS1 + phases
B1/C1 + LSTM steps 0-7 all overlap RS2. Global v columns are laid out
h-major (col = lhalf*1024 + block*128 + (l%8)*16 + b) so each half streams
contiguously into a single 40KB/partition SBUF buffer (half 1 re-streams
during half 0's matmuls), which frees room to preload W_ih/W_hh CONCURRENTLY
with phase A — phase C never waits on weight DMA. DMA queues: sync = v/wd
stream then W_ih/W_hh (ordered behind it so the stream is never starved),
scalar = consts + collective bounce traffic, gpsimd = the collectives.

Downstream stays batch-parallel (16 batches/core) in T layout
(feature-on-partition). W_hh is fp8e4m3 (lhsT fp8 x rhs fp16 is legal):
~4e-3 total error vs the 2e-2 budget, 4 MB less DMA, and the LSTM's
per-step 256 weight-tile reload gets the faster fp8 fast-weight-load path.
LSTM elementwise runs on DVE with activations on ACT (gpsimd Q7 ucode ops
are not supported by this NEFF path), and tanh(c)/h retire in 64-col halves
so the next step's first k-tiles start early.
"""

import numpy as np
import ml_dtypes

import concourse.bass as bass
import concourse.bacc as bacc
import concourse.tile as tile
from concourse import mybir
from concourse.bass_utils import run_bass_kernel_spmd

F32 = mybir.dt.float32
F16 = mybir.dt.float16
F8 = mybir.dt.float8e4
NP_F8 = ml_dtypes.float8_e4m3

B, L, N = 128, 16, 20000
HIN, H, HT = 512, 1024, 128
G = 4 * H                     # 4096 gate rows
NCORES = 8
BLOC = B // NCORES            # 16 batches per core
R = BLOC * L                  # 256 cols per batch block (l-major)
RH = R // 2                   # 128 cols per l-half
RHT = RH * NCORES             # 1024 global cols per l-half
RTOT = B * L                  # 2048 global cols
P = 128
NSH = N // NCORES             # 2500 contraction rows per core
KT = 20                       # k-tiles per core (2500 -> 2560 padded)
NPAD = KT * P                 # 2560
DT = HIN // P                 # 4 vd feature tiles
KVT = (HIN + HT) // P         # 5 vt contraction tiles
MC = G // P                   # 32 gate row-tiles
KC = HIN // P                 # 4 xg contraction tiles
KH = H // P                   # 8 lstm contraction tiles
HH = HT // 2                  # 64-col halves of the lstm state

_CACHE = {}


def _build(phases=("A", "B", "C", "L"), null=False, reps=1, cache=True):
    """Build + compile the SPMD Bass module once.

    phases/null/reps are ablation & timing knobs for performance experiments;
    the graded path always builds the full kernel with reps=1.
    """
    key = (tuple(phases), null, reps)
    if cache and key in _CACHE:
        return _CACHE[key]

    nc = bacc.Bacc("TRN2", target_bir_lowering=False, debug=False,
                   num_devices=NCORES)

    d_vT = nc.dram_tensor("vT", [NPAD, RTOT], F16, kind="ExternalInput")
    d_WdT = nc.dram_tensor("WdT", [NPAD, HIN], F16, kind="ExternalInput")
    d_WvtT = nc.dram_tensor("WvtT", [HIN + HT, HIN], F16, kind="ExternalInput")
    d_WihT = nc.dram_tensor("WihT", [HIN, G], F16, kind="ExternalInput")
    d_WhhT = nc.dram_tensor("WhhT8", [H, G], F8, kind="ExternalInput")
    d_t = nc.dram_tensor("t_row", [1, R], F32, kind="ExternalInput")
    d_wt = nc.dram_tensor("wt_row", [1, HT], F32, kind="ExternalInput")
    d_wtb = nc.dram_tensor("wtb_row", [1, HT], F32, kind="ExternalInput")
    d_bias = nc.dram_tensor("bias_g", [P, MC], F32, kind="ExternalInput")
    d_linw = nc.dram_tensor("lin_wT", [P, KH], F16, kind="ExternalInput")
    d_linb = nc.dram_tensor("lin_b_col", [BLOC, 1], F32, kind="ExternalInput")
    d_pred = nc.dram_tensor("pred", [BLOC, 1], F32, kind="ExternalOutput")

    SIG = mybir.ActivationFunctionType.Sigmoid
    TANH = mybir.ActivationFunctionType.Tanh
    IDENT = mybir.ActivationFunctionType.Identity

    with tile.TileContext(nc) as tc:
        with (
            tc.tile_pool(name="const", bufs=1) as const,
            tc.tile_pool(name="ws", bufs=2) as ws,
            tc.tile_pool(name="h16", bufs=2) as h16pool,
            tc.tile_pool(name="dram", bufs=1, space="DRAM") as dram,
        ):
            if null:
                linb_sb = const.tile([BLOC, 1], F32)
                nc.sync.dma_start(out=linb_sb, in_=d_linb[:])
                pred_sb = const.tile([BLOC, 1], F32)
                nc.scalar.copy(out=pred_sb, in_=linb_sb)
                nc.sync.dma_start(out=d_pred[:], in_=pred_sb)
            else:
                _emit(nc, tc, const, ws, h16pool, dram, phases, reps,
                      d_vT, d_WdT, d_WvtT, d_WihT, d_WhhT, d_t,
                      d_wt, d_wtb, d_bias, d_linw, d_linb, d_pred,
                      SIG, TANH, IDENT)

    nc.compile()
    if cache:
        _CACHE[key] = nc
    return nc


def _emit(nc, tc, const, ws, h16pool, dram, phases, reps,
          d_vT, d_WdT, d_WvtT, d_WihT, d_WhhT, d_t, d_wt, d_wtb,
          d_bias, d_linw, d_linb, d_pred, SIG, TANH, IDENT):
    te_sb = const.tile([P, R], F16)            # time embedding, fp16
    vd2_sb = const.tile([P, DT * R], F16)      # reduced vdT for our block
    inpT_sb = const.tile([P, KC * R], F16)     # inputsT
    xgT_sb = const.tile([P, MC * R], F16)      # 16KB/part
    cT = const.tile([P, HT], F32)              # cell state [128, 8*16]
    wvt_sb = const.tile([P, KVT * HIN], F16)   # 5KB/part
    bias_sb = const.tile([P, MC], F32)
    linw_sb = const.tile([P, KH], F16)
    linb_sb = const.tile([BLOC, 1], F32)
    t_sb = const.tile([1, R], F32)
    wt_sb = const.tile([1, HT], F32)
    wtb_sb = const.tile([1, HT], F32)
    ones_sb = const.tile([1, R], F32)

    # collective bounce buffers (DRAM), one pair per l-half. Layout
    # [j, p, d, c]: the (d, c) tail is 1KB-contiguous per partition for fast
    # DMA, and rank j's flat ReduceScatter shard is exactly [p, d, c].
    in_bs = [dram.tile([NCORES, P, DT, RH], F16, name=f"in_b{h}")
             for h in range(2)]
    out_bs = [dram.tile([P, DT, RH], F16, name=f"out_b{h}")
              for h in range(2)]

    wdT3 = d_WdT.rearrange("(a p) d -> p a d", p=P)   # [128, KT, 512]
    vT4 = d_vT.rearrange("(a p) (h q) -> p a h q", p=P, h=2)
    xg3 = xgT_sb.rearrange("p (m r) -> p m r", m=MC)  # [128, 32, 256]
    vd23 = vd2_sb.rearrange("p (d c) -> p d c", d=DT)

    # small constants on the vector DMA queue (keeps sync free for v/wd)
    for k in range(KVT):
        nc.scalar.dma_start(out=wvt_sb[:, k * HIN:(k + 1) * HIN],
                            in_=d_WvtT[k * P:(k + 1) * P, :])
    nc.scalar.dma_start(out=bias_sb, in_=d_bias[:])
    nc.scalar.dma_start(out=linw_sb, in_=d_linw[:])
    nc.scalar.dma_start(out=linb_sb, in_=d_linb[:])
    nc.scalar.dma_start(out=t_sb, in_=d_t[:])
    nc.scalar.dma_start(out=wt_sb, in_=d_wt[:])
    nc.scalar.dma_start(out=wtb_sb, in_=d_wtb[:])
    nc.vector.memset(ones_sb, 1.0)

    for rep in range(reps):
        # ---- LSTM/C weights: preloaded concurrently with phase A; they
        # ride the sync queue BEHIND the phase-A stream so they never
        # starve it (wih lands ~47us, whh ~58us, both before first use)
        wpool_cm = tc.tile_pool(name="wpool", bufs=1)
        wpool = wpool_cm.__enter__()
        whh_sb = wpool.tile([P, KH * G], F8, tag="whh_sb",
                            name="whh_sb")                         # 32KB/part
        wih_sb = wpool.tile([P, KC * G], F16, tag="wih_sb",
                            name="wih_sb")                         # 32KB/part

        # ---- phase A: partial vdT[d, r] += WdT[n,d].T @ vT[n,r], all r ----
        with (
            tc.tile_pool(name="vpool", bufs=1) as vpool,
            tc.tile_pool(name="psa", bufs=1, space="PSUM") as psa,
        ):
            wd_res = vpool.tile([P, KT, HIN], F16, tag="wd_res",
                                name="wd_res")                      # 20KB/part
            # one l-half of v resident at a time (40KB/part); half 1
            # re-streams into the same buffer behind half 0's matmuls
            v_res = vpool.tile([P, KT, RHT], F16, tag="v_res",
                               name="v_res")
            ksizes = [2] * (KT // 2)
            for half in range(2 if "A" in phases else 0):
                k0 = 0
                for sz in ksizes:
                    if half == 0:
                        nc.sync.dma_start(out=wd_res[:, k0:k0 + sz, :],
                                          in_=wdT3[:, k0:k0 + sz, :])
                    nc.sync.dma_start(out=v_res[:, k0:k0 + sz, :],
                                      in_=vT4[:, k0:k0 + sz, half, :])
                    k0 += sz
                if half == 1:
                    # weights queue behind the full v stream
                    for k in range(KC):
                        nc.sync.dma_start(out=wih_sb[:, k * G:(k + 1) * G],
                                          in_=d_WihT[k * P:(k + 1) * P, :])
                    for k in range(KH):
                        nc.sync.dma_start(out=whh_sb[:, k * G:(k + 1) * G],
                                          in_=d_WhhT[k * P:(k + 1) * P, :])
                psA = [psa.tile([P, RHT], F32, tag=f"psA{d}",
                                name=f"psA{d}_{half}") for d in range(DT)]
                for k in range(KT):
                    for d in range(DT):
                        for nb in range(2):
                            nc.tensor.matmul(
                                psA[d][:, nb * 512:(nb + 1) * 512],
                                lhsT=wd_res[:, k, d * P:(d + 1) * P],
                                rhs=v_res[:, k, nb * 512:(nb + 1) * 512],
                                start=(k == 0), stop=(k == KT - 1))
                # stage fp16 partials j-major so the bounce DMA collapses
                # to 3 dims: vdp cols = j*(DT*128) + d*128 + c
                vdp = vpool.tile([P, DT * RHT], F16, tag="vdp",
                                 name=f"vdp_{half}", bufs=2)
                vdp4 = vdp.rearrange("p (j d c) -> p j d c", j=NCORES, d=DT)
                eng = [nc.vector, nc.scalar, nc.vector, nc.vector]
                for d in range(DT):
                    psA3 = psA[d].rearrange("p (j c) -> p j c", j=NCORES)
                    if d == 1:
                        nc.scalar.copy(out=vdp4[:, :, d, :], in_=psA3)
                    else:
                        eng[d].tensor_copy(out=vdp4[:, :, d, :], in_=psA3)
                nc.scalar.dma_start(
                    out=in_bs[half].rearrange("j p d c -> p j (d c)"),
                    in_=vdp.rearrange("p (j dc) -> p j dc", j=NCORES))
                nc.gpsimd.collective_compute(
                    "ReduceScatter", mybir.AluOpType.add,
                    replica_groups=[list(range(NCORES))],
                    ins=[in_bs[half].opt()], outs=[out_bs[half].opt()])
                # readback: our block's columns for this l-half
                nc.scalar.dma_start(
                    out=vd23[:, :, half * RH:(half + 1) * RH],
                    in_=out_bs[half].rearrange("p d c -> p d c"))

        with tc.tile_pool(name="psmm", bufs=1, space="PSUM") as psmm:
            # te: wt[j] * t[r] + wtb[j]  (rank-1 matmuls, K=1)
            psTE = psmm.tile([P, R], F32, tag="psTE", name="psTE")
            nc.tensor.matmul(psTE, lhsT=wt_sb, rhs=t_sb, start=True,
                             stop=False)
            nc.tensor.matmul(psTE, lhsT=wtb_sb, rhs=ones_sb, start=False,
                             stop=True)
            nc.vector.tensor_copy(out=te_sb, in_=psTE)

            h_prev = None
            for lh in range(2):
                c0 = lh * RH
                # ---- phase B (this l-half): inputsT over [vd2; te] ----
                for m in range(KC if "B" in phases else 0):
                    psB = psmm.tile([P, RH], F32, tag="psBC",
                                    name=f"psB{m}_{lh}", bufs=3)
                    for k in range(KVT):
                        rhs = (vd23[:, k, c0:c0 + RH] if k < KC
                               else te_sb[:, c0:c0 + RH])
                        nc.tensor.matmul(
                            psB,
                            lhsT=wvt_sb[:, k * HIN + m * P:
                                        k * HIN + (m + 1) * P],
                            rhs=rhs, start=(k == 0), stop=(k == KVT - 1))
                    nc.vector.tensor_copy(
                        out=inpT_sb[:, m * R + c0:m * R + c0 + RH], in_=psB)

                # ---- phase C (this l-half): xgT = WihT.T @ inputsT + b ----
                for m in range(MC if "C" in phases else 0):
                    psC = psmm.tile([P, RH], F32, tag="psBC",
                                    name=f"psC{m}_{lh}", bufs=3)
                    for k in range(KC):
                        nc.tensor.matmul(
                            psC,
                            lhsT=wih_sb[:, k * G + m * P:k * G + (m + 1) * P],
                            rhs=inpT_sb[:, k * R + c0:k * R + c0 + RH],
                            start=(k == 0), stop=(k == KC - 1))
                    if m % 2 == 0:
                        nc.scalar.activation(
                            out=xgT_sb[:, m * R + c0:m * R + c0 + RH],
                            in_=psC, func=IDENT,
                            bias=bias_sb[:, m:m + 1], scale=1.0)
                    else:
                        nc.vector.tensor_scalar_add(
                            xgT_sb[:, m * R + c0:m * R + c0 + RH],
                            psC, bias_sb[:, m:m + 1])

                # ---- LSTM steps of this l-half ----
                for t in range(lh * 8, (lh + 1) * 8 if "L" in phases else 0):
                    acts = [None] * 4
                    if t == 0:
                        # f-gate is dead at t=0 (c0 = 0); gates = xg directly
                        for g in (0, 2, 3):
                            a = ws.tile([P, HT], F32, tag=f"act{g}",
                                        name=f"act{g}_{t}")
                            a3 = a.rearrange("p (m j) -> p m j", m=KH)
                            nc.scalar.activation(
                                out=a3, in_=xg3[:, g * KH:(g + 1) * KH,
                                                t * BLOC:(t + 1) * BLOC],
                                func=(TANH if g == 2 else SIG))
                            acts[g] = a
                    else:
                        # two psum banks: (i,g) then (f,o). pre-adds split
                        # DVE/gpsimd; activations on ACT
                        pshs = [psmm.tile([P, 2 * HT], F32, tag=f"psL{b}",
                                          name=f"psL{b}_{t}", bufs=2)
                                for b in range(2)]
                        order = [0, 2, 1, 3]           # i, g, f, o
                        for gi, g in enumerate(order):
                            b, pos = divmod(gi, 2)
                            ph = pshs[b]
                            for mt in range(KH):
                                m = g * KH + mt
                                mm = pos * KH + mt
                                sl = slice(mm * BLOC, (mm + 1) * BLOC)
                                for k in range(KH):
                                    hp = h_prev[k // 4]
                                    nc.tensor.matmul(
                                        ph[:, sl],
                                        lhsT=whh_sb[:, k * G + m * P:
                                                    k * G + (m + 1) * P],
                                        rhs=hp[:, (k % 4) * BLOC:
                                               (k % 4 + 1) * BLOC],
                                        start=(k == 0), stop=(k == KH - 1))
                            ph3 = ph.rearrange("p (m j) -> p m j", m=MC // 2)
                            pre = ws.tile([P, HT], F32, tag=f"pre{g}",
                                          name=f"pre{g}_{t}")
                            pre3 = pre.rearrange("p (m j) -> p m j", m=KH)
                            peng = nc.vector
                            peng.tensor_add(
                                pre3,
                                ph3[:, pos * KH:(pos + 1) * KH, :],
                                xg3[:, g * KH:(g + 1) * KH,
                                    t * BLOC:(t + 1) * BLOC])
                            a = ws.tile([P, HT], F32, tag=f"act{g}",
                                        name=f"act{g}_{t}")
                            nc.scalar.activation(out=a, in_=pre,
                                                 func=(TANH if g == 2
                                                       else SIG))
                            acts[g] = a
                    i_a, f_a, g_a, o_a = acts
                    # c update full-width: ig on DVE, fc on gpsimd
                    ig = ws.tile([P, HT], F32, tag="ig", name=f"ig_{t}")
                    nc.vector.tensor_mul(ig, i_a, g_a)
                    if t == 0:
                        nc.vector.tensor_copy(out=cT, in_=ig)
                    else:
                        fc = ws.tile([P, HT], F32, tag="fc", name=f"fc_{t}")
                        nc.vector.tensor_mul(fc, f_a, cT)
                        nc.vector.tensor_add(cT, ig, fc)
                    # tanh + h in 64-col halves so next step's first k-tiles
                    # start while the second half retires
                    h_new = [None, None]
                    for hh in range(2):
                        sl = slice(hh * HH, (hh + 1) * HH)
                        tc_t = ws.tile([P, HH], F32, tag=f"tanhc{hh}",
                                       name=f"tanhc{hh}_{t}")
                        nc.scalar.activation(out=tc_t, in_=cT[:, sl],
                                             func=TANH)
                        hn = h16pool.tile([P, HH], F16, tag=f"h16{hh}",
                                          name=f"h16{hh}_{t}")
                        nc.vector.tensor_mul(hn, o_a[:, sl], tc_t)
                        h_new[hh] = hn
                    h_prev = h_new

            # ---- head: pred = hT.T @ lin_wT + lin_b ----
            if h_prev is None:
                h_prev = [h16pool.tile([P, HH], F16, tag=f"h16{hh}",
                                       name=f"h16_d{hh}") for hh in range(2)]
                for hh in range(2):
                    nc.vector.memset(h_prev[hh], 0.0)
            psP = psmm.tile([BLOC, 1], F32, tag="psTE", name="psP")
            for k in range(KH):
                hp = h_prev[k // 4]
                nc.tensor.matmul(psP,
                                 lhsT=hp[:, (k % 4) * BLOC:
                                        (k % 4 + 1) * BLOC],
                                 rhs=linw_sb[:, k:k + 1],
                                 start=(k == 0), stop=(k == KH - 1))
            pred_sb = const.tile([BLOC, 1], F32, name="pred_sb",
                                 tag="pred_sb")
            nc.scalar.activation(out=pred_sb, in_=psP, func=IDENT,
                                 bias=linb_sb, scale=1.0)
            nc.sync.dma_start(out=d_pred[:], in_=pred_sb)
        wpool_cm.__exit__(None, None, None)


def _prep_in_maps(v, t, W_down, Wt_up_w, Wt_up_b, W_vt, W_ih, W_hh,
                  b_ih, b_hh, lin_w, lin_b):
    """Host-side shard/layout/dtype prep. Layout + cast only, no math."""
    WvtT = np.ascontiguousarray(W_vt.T).astype(np.float16)
    WihT = np.ascontiguousarray(W_ih.T).astype(np.float16)
    WhhT8 = np.ascontiguousarray(W_hh.T).astype(NP_F8)
    wt_row = np.ascontiguousarray(Wt_up_w.reshape(1, HT)).astype(np.float32)
    wtb_row = np.ascontiguousarray(Wt_up_b.reshape(1, HT)).astype(np.float32)
    bias_g = np.ascontiguousarray(
        (b_ih + b_hh).astype(np.float32).reshape(MC, P).T)
    lin_wT = np.ascontiguousarray(
        lin_w.reshape(KH, P).T).astype(np.float16)
    lin_b_col = np.full((BLOC, 1), np.float32(lin_b[0]), np.float32)

    # global columns h-major: col = (l//8)*1024 + bblock*128 + (l%8)*16 + b
    vg = np.ascontiguousarray(
        v.reshape(NCORES, BLOC, 2, L // 2, N)
        .transpose(4, 2, 0, 3, 1).reshape(N, RTOT))

    shared = dict(WvtT=WvtT, WihT=WihT, WhhT8=WhhT8, wt_row=wt_row,
                  wtb_row=wtb_row, bias_g=bias_g, lin_wT=lin_wT,
                  lin_b_col=lin_b_col)
    in_maps = []
    for c in range(NCORES):
        n0 = c * NSH
        vT = np.zeros((NPAD, RTOT), np.float16)
        vT[:NSH] = vg[n0:n0 + NSH].astype(np.float16)
        WdT = np.zeros((NPAD, HIN), np.float16)
        WdT[:NSH] = W_down[:, n0:n0 + NSH].T.astype(np.float16)
        b0 = c * BLOC
        t_row = np.ascontiguousarray(
            t[b0:b0 + BLOC].T.reshape(1, R)).astype(np.float32)
        in_maps.append(dict(vT=vT, WdT=WdT, t_row=t_row, **shared))
    return in_maps


def kernel(**inputs):
    nc = _build()
    in_maps = _prep_in_maps(**inputs)
    res = run_bass_kernel_spmd(nc, in_maps, core_ids=list(range(NCORES)))
    return np.concatenate([res.results[c]["pred"] for c in range(NCORES)],
                          axis=0).astype(np.float32)



# revision 19
# speedup vs baseline: 1.3219x; 1.3219x over previous
"""Trainium2 Bass kernel for nn_Discriminator (W_down projection + time-embed
+ W_vt/W_ih projections + 16-step LSTM + linear head).

Strategy: phase A (the dominant 42-GFLOP W_down contraction) is sharded over
the CONTRACTION dim N across the 8 cores: core k holds W_down columns
[k*2500, (k+1)*2500) (2.6 MB fp16 instead of 20.6 MB replicated) and the
matching slice of v for ALL batches. Each core computes partial
vdT [512, 2048] for the full batch; fp16 ReduceScatters (CCE adds on the
SDMA path, compute engines stay free) hand core j the reduced vdT [512, 256]
for its batch block. This turns phase A from DMA-bound (43 MB/core) into
compute-bound (~68 us).

The reduction is split into two collectives by l-half (columns l<8 / l>=8 of
every batch block): phase A computes the l<8 columns first, so RS1 + phases
B1/C1 + LSTM steps 0-7 all overlap RS2. Within each half the psum is retired
per j-half (two 4-bank passes over the k range), so the psum->SBUF copies
and the bounce DMA for target cores 0-3 start at the midpoint of the half's
matmul stream — RS1 issues ~8us earlier and hides fully behind phase A's
half-1 matmuls. Global v columns are laid out h-major (col = lhalf*1024 +
block*128 + (l%8)*16 + b) so each half streams contiguously into a single
40KB/partition SBUF buffer (half 1 re-streams during half 0's matmuls),
which frees room to preload W_ih/W_hh CONCURRENTLY with phase A — phase C
never waits on weight DMA. DMA queues: sync = v/wd stream (single k-tile
chunks up front so the first matmuls start early) then W_ih/W_hh in fine
chunks (so bounce DMAs never queue long behind them), scalar = consts +
collective bounce traffic, gpsimd = the collectives.

Downstream stays batch-parallel (16 batches/core) in T layout
(feature-on-partition). W_hh is fp8e4m3 (lhsT fp8 x rhs fp16 is legal):
~4e-3 total error vs the 2e-2 budget, 4 MB less DMA, and the LSTM's
per-step 256 weight-tile reload gets the faster fp8 fast-weight-load path.
LSTM elementwise runs on DVE with activations on ACT (gpsimd Q7 ucode ops
are not supported by this NEFF path). Per step the gates stream as psum
banks (f,g) and (i,o): the scheduler retires pos0 of both banks first, so f
lands first and o last — fc/ig/c/tanh(c) hide under o's matmuls and the
post-stream tail is only sigmoid(o) + h-mul, retired in 64-col halves so
the next step's first k-tiles start early.
"""

import numpy as np
import ml_dtypes

import concourse.bass as bass
import concourse.bacc as bacc
import concourse.tile as tile
from concourse import mybir
from concourse.bass_utils import run_bass_kernel_spmd

F32 = mybir.dt.float32
F16 = mybir.dt.float16
F8 = mybir.dt.float8e4
NP_F8 = ml_dtypes.float8_e4m3

B, L, N = 128, 16, 20000
HIN, H, HT = 512, 1024, 128
G = 4 * H                     # 4096 gate rows
NCORES = 8
BLOC = B // NCORES            # 16 batches per core
R = BLOC * L                  # 256 cols per batch block (l-major)
RH = R // 2                   # 128 cols per l-half
RHT = RH * NCORES             # 1024 global cols per l-half
RTOT = B * L                  # 2048 global cols
P = 128
NSH = N // NCORES             # 2500 contraction rows per core
KT = 20                       # k-tiles per core (2500 -> 2560 padded)
NPAD = KT * P                 # 2560
DT = HIN // P                 # 4 vd feature tiles
KVT = (HIN + HT) // P         # 5 vt contraction tiles
MC = G // P                   # 32 gate row-tiles
KC = HIN // P                 # 4 xg contraction tiles
KH = H // P                   # 8 lstm contraction tiles
HH = HT // 2                  # 64-col halves of the lstm state

_CACHE = {}


def _build(phases=("A", "B", "C", "L"), null=False, reps=1, cache=True):
    """Build + compile the SPMD Bass module once.

    phases/null/reps are ablation & timing knobs for performance experiments;
    the graded path always builds the full kernel with reps=1.
    """
    key = (tuple(phases), null, reps)
    if cache and key in _CACHE:
        return _CACHE[key]

    nc = bacc.Bacc("TRN2", target_bir_lowering=False, debug=False,
                   num_devices=NCORES)

    d_vT = nc.dram_tensor("vT", [NPAD, RTOT], F16, kind="ExternalInput")
    d_WdT = nc.dram_tensor("WdT", [NPAD, HIN], F16, kind="ExternalInput")
    d_WvtT = nc.dram_tensor("WvtT", [HIN + HT, HIN], F16, kind="ExternalInput")
    d_WihT = nc.dram_tensor("WihT", [HIN, G], F16, kind="ExternalInput")
    d_WhhT = nc.dram_tensor("WhhT8", [H, G], F8, kind="ExternalInput")
    d_t = nc.dram_tensor("t_row", [1, R], F32, kind="ExternalInput")
    d_wt = nc.dram_tensor("wt_row", [1, HT], F32, kind="ExternalInput")
    d_wtb = nc.dram_tensor("wtb_row", [1, HT], F32, kind="ExternalInput")
    d_bias = nc.dram_tensor("bias_g", [P, MC], F32, kind="ExternalInput")
    d_linw = nc.dram_tensor("lin_wT", [P, KH], F16, kind="ExternalInput")
    d_linb = nc.dram_tensor("lin_b_col", [BLOC, 1], F32, kind="ExternalInput")
    d_pred = nc.dram_tensor("pred", [BLOC, 1], F32, kind="ExternalOutput")

    SIG = mybir.ActivationFunctionType.Sigmoid
    TANH = mybir.ActivationFunctionType.Tanh
    IDENT = mybir.ActivationFunctionType.Identity

    with tile.TileContext(nc) as tc:
        with (
            tc.tile_pool(name="const", bufs=1) as const,
            tc.tile_pool(name="ws", bufs=2) as ws,
            tc.tile_pool(name="h16", bufs=2) as h16pool,
            tc.tile_pool(name="dram", bufs=1, space="DRAM") as dram,
        ):
            if null:
                linb_sb = const.tile([BLOC, 1], F32)
                nc.sync.dma_start(out=linb_sb, in_=d_linb[:])
                pred_sb = const.tile([BLOC, 1], F32)
                nc.scalar.copy(out=pred_sb, in_=linb_sb)
                nc.sync.dma_start(out=d_pred[:], in_=pred_sb)
            else:
                _emit(nc, tc, const, ws, h16pool, dram, phases, reps,
                      d_vT, d_WdT, d_WvtT, d_WihT, d_WhhT, d_t,
                      d_wt, d_wtb, d_bias, d_linw, d_linb, d_pred,
                      SIG, TANH, IDENT)

    nc.compile()
    if cache:
        _CACHE[key] = nc
    return nc


def _emit(nc, tc, const, ws, h16pool, dram, phases, reps,
          d_vT, d_WdT, d_WvtT, d_WihT, d_WhhT, d_t, d_wt, d_wtb,
          d_bias, d_linw, d_linb, d_pred, SIG, TANH, IDENT):
    te_sb = const.tile([P, R], F16)            # time embedding, fp16
    vd2_sb = const.tile([P, DT * R], F16)      # reduced vdT for our block
    inpT_sb = const.tile([P, KC * R], F16)     # inputsT
    xgT_sb = const.tile([P, MC * R], F16)      # 16KB/part
    cT = const.tile([P, HT], F32)              # cell state [128, 8*16]
    wvt_sb = const.tile([P, KVT * HIN], F16)   # 5KB/part
    bias_sb = const.tile([P, MC], F32)
    linw_sb = const.tile([P, KH], F16)
    linb_sb = const.tile([BLOC, 1], F32)
    t_sb = const.tile([1, R], F32)
    wt_sb = const.tile([1, HT], F32)
    wtb_sb = const.tile([1, HT], F32)
    ones_sb = const.tile([1, R], F32)

    # collective bounce buffers (DRAM), one pair per l-half. Layout
    # [j, p, d, c]: the (d, c) tail is 1KB-contiguous per partition for fast
    # DMA, and rank j's flat ReduceScatter shard is exactly [p, d, c].
    in_bs = [dram.tile([NCORES, P, DT, RH], F16, name=f"in_b{h}")
             for h in range(2)]
    out_bs = [dram.tile([P, DT, RH], F16, name=f"out_b{h}")
              for h in range(2)]

    wdT3 = d_WdT.rearrange("(a p) d -> p a d", p=P)   # [128, KT, 512]
    vT4 = d_vT.rearrange("(a p) (h q) -> p a h q", p=P, h=2)
    xg3 = xgT_sb.rearrange("p (m r) -> p m r", m=MC)  # [128, 32, 256]
    vd23 = vd2_sb.rearrange("p (d c) -> p d c", d=DT)

    # small constants on the vector DMA queue (keeps sync free for v/wd)
    for k in range(KVT):
        nc.scalar.dma_start(out=wvt_sb[:, k * HIN:(k + 1) * HIN],
                            in_=d_WvtT[k * P:(k + 1) * P, :])
    nc.scalar.dma_start(out=bias_sb, in_=d_bias[:])
    nc.scalar.dma_start(out=linw_sb, in_=d_linw[:])
    nc.scalar.dma_start(out=linb_sb, in_=d_linb[:])
    nc.scalar.dma_start(out=t_sb, in_=d_t[:])
    nc.scalar.dma_start(out=wt_sb, in_=d_wt[:])
    nc.scalar.dma_start(out=wtb_sb, in_=d_wtb[:])
    nc.vector.memset(ones_sb, 1.0)

    for rep in range(reps):
        # ---- LSTM/C weights: preloaded concurrently with phase A; they
        # ride the sync queue BEHIND the phase-A stream so they never
        # starve it (wih lands ~47us, whh ~58us, both before first use)
        wpool_cm = tc.tile_pool(name="wpool", bufs=1)
        wpool = wpool_cm.__enter__()
        whh_sb = wpool.tile([P, KH * G], F8, tag="whh_sb",
                            name="whh_sb")                         # 32KB/part
        wih_sb = wpool.tile([P, KC * G], F16, tag="wih_sb",
                            name="wih_sb")                         # 32KB/part

        # ---- phase A: partial vdT[d, r] += WdT[n,d].T @ vT[n,r], all r ----
        with (
            tc.tile_pool(name="vpool", bufs=1) as vpool,
            tc.tile_pool(name="psa", bufs=1, space="PSUM") as psa,
        ):
            wd_res = vpool.tile([P, KT, HIN], F16, tag="wd_res",
                                name="wd_res")                      # 20KB/part
            # one l-half of v resident at a time (40KB/part); half 1
            # re-streams into the same buffer behind half 0's matmuls
            v_res = vpool.tile([P, KT, RHT], F16, tag="v_res",
                               name="v_res")
            # single k-tile chunks up front so the first matmuls start
            # ~1.5us earlier; pairs after that for DMA efficiency
            ksizes = [1, 1, 1, 1] + [2] * ((KT - 4) // 2)
            for half in range(2 if "A" in phases else 0):
                k0 = 0
                for sz in ksizes:
                    if half == 0:
                        nc.sync.dma_start(out=wd_res[:, k0:k0 + sz, :],
                                          in_=wdT3[:, k0:k0 + sz, :])
                    nc.sync.dma_start(out=v_res[:, k0:k0 + sz, :],
                                      in_=vT4[:, k0:k0 + sz, half, :])
                    k0 += sz
                if half == 1:
                    # weights queue behind the full v stream; fine chunks so
                    # the half-0 bounce DMA never waits long behind them
                    for k in range(KC):
                        for c in range(2):
                            nc.sync.dma_start(
                                out=wih_sb[:, k * G + c * (G // 2):
                                           k * G + (c + 1) * (G // 2)],
                                in_=d_WihT[k * P:(k + 1) * P,
                                           c * (G // 2):(c + 1) * (G // 2)])
                    for k in range(KH):
                        nc.sync.dma_start(out=whh_sb[:, k * G:(k + 1) * G],
                                          in_=d_WhhT[k * P:(k + 1) * P, :])
                # psum retired per j-half: copies + bounce of target cores
                # 0-3 start at the midpoint of this half's matmul stream
                psA = [[psa.tile([P, RHT // 2], F32, tag=f"psA{d}{jh}",
                                 name=f"psA{d}{jh}_{half}") for d in range(DT)]
                       for jh in range(2)]
                # stage fp16 partials j-major so the bounce DMA collapses
                # to 3 dims: vdp cols = j*(DT*128) + d*128 + c
                vdp = vpool.tile([P, DT * RHT], F16, tag="vdp",
                                 name=f"vdp_{half}", bufs=2)
                vdp4 = vdp.rearrange("p (j d c) -> p j d c", j=NCORES, d=DT)
                vdp3 = vdp.rearrange("p (j dc) -> p j dc", j=NCORES)
                in_b3 = in_bs[half].rearrange("j p d c -> p j (d c)")
                for jh in range(2):
                    for k in range(KT):
                        for d in range(DT):
                            nc.tensor.matmul(
                                psA[jh][d],
                                lhsT=wd_res[:, k, d * P:(d + 1) * P],
                                rhs=v_res[:, k, jh * 512:(jh + 1) * 512],
                                start=(k == 0), stop=(k == KT - 1))
                    for d in range(DT):
                        psA3 = psA[jh][d].rearrange("p (j c) -> p j c",
                                                    j=NCORES // 2)
                        dst = vdp4[:, jh * 4:(jh + 1) * 4, d, :]
                        if d % 2 == 1:
                            nc.scalar.copy(out=dst, in_=psA3)
                        else:
                            nc.vector.tensor_copy(out=dst, in_=psA3)
                    nc.scalar.dma_start(
                        out=in_b3[:, jh * 4:(jh + 1) * 4, :],
                        in_=vdp3[:, jh * 4:(jh + 1) * 4, :])
                nc.gpsimd.collective_compute(
                    "ReduceScatter", mybir.AluOpType.add,
                    replica_groups=[list(range(NCORES))],
                    ins=[in_bs[half].opt()], outs=[out_bs[half].opt()])
                # readback: our block's columns for this l-half
                nc.scalar.dma_start(
                    out=vd23[:, :, half * RH:(half + 1) * RH],
                    in_=out_bs[half].rearrange("p d c -> p d c"))

        with tc.tile_pool(name="psmm", bufs=1, space="PSUM") as psmm:
            # te: wt[j] * t[r] + wtb[j]  (rank-1 matmuls, K=1)
            psTE = psmm.tile([P, R], F32, tag="psTE", name="psTE")
            nc.tensor.matmul(psTE, lhsT=wt_sb, rhs=t_sb, start=True,
                             stop=False)
            nc.tensor.matmul(psTE, lhsT=wtb_sb, rhs=ones_sb, start=False,
                             stop=True)
            nc.vector.tensor_copy(out=te_sb, in_=psTE)

            h_prev = None
            for lh in range(2):
                c0 = lh * RH
                # B/C of the first l-half gate the LSTM's serial chain:
                # raise their priority so they preempt phase A's half-1
                # matmul stream the moment the RS readback lands
                prio_cm = tc.high_priority() if lh == 0 else None
                if prio_cm is not None:
                    prio_cm.__enter__()
                # ---- phase B (this l-half): inputsT over [vd2; te] ----
                for m in range(KC if "B" in phases else 0):
                    psB = psmm.tile([P, RH], F32, tag="psBC",
                                    name=f"psB{m}_{lh}", bufs=3)
                    for k in range(KVT):
                        rhs = (vd23[:, k, c0:c0 + RH] if k < KC
                               else te_sb[:, c0:c0 + RH])
                        nc.tensor.matmul(
                            psB,
                            lhsT=wvt_sb[:, k * HIN + m * P:
                                        k * HIN + (m + 1) * P],
                            rhs=rhs, start=(k == 0), stop=(k == KVT - 1))
                    nc.vector.tensor_copy(
                        out=inpT_sb[:, m * R + c0:m * R + c0 + RH], in_=psB)

                # ---- phase C (this l-half): xgT = WihT.T @ inputsT + b ----
                for m in range(MC if "C" in phases else 0):
                    psC = psmm.tile([P, RH], F32, tag="psBC",
                                    name=f"psC{m}_{lh}", bufs=3)
                    for k in range(KC):
                        nc.tensor.matmul(
                            psC,
                            lhsT=wih_sb[:, k * G + m * P:k * G + (m + 1) * P],
                            rhs=inpT_sb[:, k * R + c0:k * R + c0 + RH],
                            start=(k == 0), stop=(k == KC - 1))
                    if m % 2 == 0:
                        nc.scalar.activation(
                            out=xgT_sb[:, m * R + c0:m * R + c0 + RH],
                            in_=psC, func=IDENT,
                            bias=bias_sb[:, m:m + 1], scale=1.0)
                    else:
                        nc.vector.tensor_scalar_add(
                            xgT_sb[:, m * R + c0:m * R + c0 + RH],
                            psC, bias_sb[:, m:m + 1])
                if prio_cm is not None:
                    prio_cm.__exit__(None, None, None)

                # ---- LSTM steps of this l-half ----
                for t in range(lh * 8, (lh + 1) * 8 if "L" in phases else 0):
                    acts = [None] * 4
                    if t == 0:
                        # f-gate is dead at t=0 (c0 = 0); gates = xg directly
                        for g in (0, 2, 3):
                            a = ws.tile([P, HT], F32, tag=f"act{g}",
                                        name=f"act{g}_{t}")
                            a3 = a.rearrange("p (m j) -> p m j", m=KH)
                            nc.scalar.activation(
                                out=a3, in_=xg3[:, g * KH:(g + 1) * KH,
                                                t * BLOC:(t + 1) * BLOC],
                                func=(TANH if g == 2 else SIG))
                            acts[g] = a
                        i_a, g_a, o_a = acts[0], acts[2], acts[3]
                        ig = ws.tile([P, HT], F32, tag="ig", name=f"ig_{t}")
                        nc.vector.tensor_mul(ig, i_a, g_a)
                        nc.vector.tensor_copy(out=cT, in_=ig)
                        h_new = [None, None]
                        for hh in range(2):
                            sl = slice(hh * HH, (hh + 1) * HH)
                            tc_t = ws.tile([P, HH], F32, tag=f"tanhc{hh}",
                                           name=f"tanhc{hh}_{t}")
                            nc.scalar.activation(out=tc_t, in_=cT[:, sl],
                                                 func=TANH)
                            hn = h16pool.tile([P, HH], F16, tag=f"h16{hh}",
                                              name=f"h16{hh}_{t}")
                            nc.vector.tensor_mul(hn, o_a[:, sl], tc_t)
                            h_new[hh] = hn
                        h_prev = h_new
                        continue
                    # psum banks (f,g) and (i,o): the scheduler retires pos0
                    # of both banks first, then pos1 — so f lands first and o
                    # last under either pattern. The c chain (fc, ig, c,
                    # tanh) hides under o's matmuls; the tail is only o's
                    # act halves + h muls.
                    pshs = [psmm.tile([P, 2 * HT], F32, tag=f"psL{b}",
                                      name=f"psL{b}_{t}", bufs=2)
                            for b in range(2)]
                    order = [1, 2, 0, 3]               # f, g, i, o
                    for gi, g in enumerate(order):
                        b, pos = divmod(gi, 2)
                        ph = pshs[b]
                        for mt in range(KH):
                            m = g * KH + mt
                            mm = pos * KH + mt
                            sl = slice(mm * BLOC, (mm + 1) * BLOC)
                            for k in range(KH):
                                hp = h_prev[k // 4]
                                nc.tensor.matmul(
                                    ph[:, sl],
                                    lhsT=whh_sb[:, k * G + m * P:
                                                k * G + (m + 1) * P],
                                    rhs=hp[:, (k % 4) * BLOC:
                                           (k % 4 + 1) * BLOC],
                                    start=(k == 0), stop=(k == KH - 1))
                        ph3 = ph.rearrange("p (m j) -> p m j", m=MC // 2)
                        pre = ws.tile([P, HT], F32, tag=f"pre{g}",
                                      name=f"pre{g}_{t}")
                        pre3 = pre.rearrange("p (m j) -> p m j", m=KH)
                        nc.vector.tensor_add(
                            pre3,
                            ph3[:, pos * KH:(pos + 1) * KH, :],
                            xg3[:, g * KH:(g + 1) * KH,
                                t * BLOC:(t + 1) * BLOC])
                        if g == 3:
                            acts[g] = pre          # activated in halves below
                            continue
                        a = ws.tile([P, HT], F32, tag=f"act{g}",
                                    name=f"act{g}_{t}")
                        nc.scalar.activation(out=a, in_=pre,
                                             func=(TANH if g == 2 else SIG))
                        acts[g] = a
                        if g == 1:                 # fc as soon as f retires
                            fc = ws.tile([P, HT], F32, tag="fc",
                                         name=f"fc_{t}")
                            nc.vector.tensor_mul(fc, a, cT)
                        elif g == 0:               # c update under o's MMs
                            ig = ws.tile([P, HT], F32, tag="ig",
                                         name=f"ig_{t}")
                            nc.vector.tensor_mul(ig, a, acts[2])
                            nc.vector.tensor_add(cT, ig, fc)
                    # tail, in 64-col halves: after o's matmuls stop only
                    # sigmoid(o) + h-mul remain; next step's first k-tiles
                    # start once h half 0 lands
                    h_new = [None, None]
                    for hh in range(2):
                        sl = slice(hh * HH, (hh + 1) * HH)
                        tc_t = ws.tile([P, HH], F32, tag=f"tanhc{hh}",
                                       name=f"tanhc{hh}_{t}")
                        nc.scalar.activation(out=tc_t, in_=cT[:, sl],
                                             func=TANH)
                        o_h = ws.tile([P, HH], F32, tag=f"oact{hh}",
                                      name=f"oact{hh}_{t}")
                        nc.scalar.activation(out=o_h, in_=acts[3][:, sl],
                                             func=SIG)
                        hn = h16pool.tile([P, HH], F16, tag=f"h16{hh}",
                                          name=f"h16{hh}_{t}")
                        nc.vector.tensor_mul(hn, o_h, tc_t)
                        h_new[hh] = hn
                    h_prev = h_new

            # ---- head: pred = hT.T @ lin_wT + lin_b ----
            if h_prev is None:
                h_prev = [h16pool.tile([P, HH], F16, tag=f"h16{hh}",
                                       name=f"h16_d{hh}") for hh in range(2)]
                for hh in range(2):
                    nc.vector.memset(h_prev[hh], 0.0)
            psP = psmm.tile([BLOC, 1], F32, tag="psTE", name="psP")
            for k in range(KH):
                hp = h_prev[k // 4]
                nc.tensor.matmul(psP,
                                 lhsT=hp[:, (k % 4) * BLOC:
                                        (k % 4 + 1) * BLOC],
                                 rhs=linw_sb[:, k:k + 1],
                                 start=(k == 0), stop=(k == KH - 1))
            pred_sb = const.tile([BLOC, 1], F32, name="pred_sb",
                                 tag="pred_sb")
            nc.scalar.activation(out=pred_sb, in_=psP, func=IDENT,
                                 bias=linb_sb, scale=1.0)
            nc.sync.dma_start(out=d_pred[:], in_=pred_sb)
        wpool_cm.__exit__(None, None, None)


def _prep_in_maps(v, t, W_down, Wt_up_w, Wt_up_b, W_vt, W_ih, W_hh,
                  b_ih, b_hh, lin_w, lin_b):
    """Host-side shard/layout/dtype prep. Layout + cast only, no math."""
    WvtT = np.ascontiguousarray(W_vt.T).astype(np.float16)
    WihT = np.ascontiguousarray(W_ih.T).astype(np.float16)
    WhhT8 = np.ascontiguousarray(W_hh.T).astype(NP_F8)
    wt_row = np.ascontiguousarray(Wt_up_w.reshape(1, HT)).astype(np.float32)
    wtb_row = np.ascontiguousarray(Wt_up_b.reshape(1, HT)).astype(np.float32)
    bias_g = np.ascontiguousarray(
        (b_ih + b_hh).astype(np.float32).reshape(MC, P).T)
    lin_wT = np.ascontiguousarray(
        lin_w.reshape(KH, P).T).astype(np.float16)
    lin_b_col = np.full((BLOC, 1), np.float32(lin_b[0]), np.float32)

    # global columns h-major: col = (l//8)*1024 + bblock*128 + (l%8)*16 + b
    vg = np.ascontiguousarray(
        v.reshape(NCORES, BLOC, 2, L // 2, N)
        .transpose(4, 2, 0, 3, 1).reshape(N, RTOT))

    shared = dict(WvtT=WvtT, WihT=WihT, WhhT8=WhhT8, wt_row=wt_row,
                  wtb_row=wtb_row, bias_g=bias_g, lin_wT=lin_wT,
                  lin_b_col=lin_b_col)
    in_maps = []
    for c in range(NCORES):
        n0 = c * NSH
        vT = np.zeros((NPAD, RTOT), np.float16)
        vT[:NSH] = vg[n0:n0 + NSH].astype(np.float16)
        WdT = np.zeros((NPAD, HIN), np.float16)
        WdT[:NSH] = W_down[:, n0:n0 + NSH].T.astype(np.float16)
        b0 = c * BLOC
        t_row = np.ascontiguousarray(
            t[b0:b0 + BLOC].T.reshape(1, R)).astype(np.float32)
        in_maps.append(dict(vT=vT, WdT=WdT, t_row=t_row, **shared))
    return in_maps


def kernel(**inputs):
    nc = _build()
    in_maps = _prep_in_maps(**inputs)
    res = run_bass_kernel_spmd(nc, in_maps, core_ids=list(range(NCORES)))
    return np.concatenate([res.results[c]["pred"] for c in range(NCORES)],
                          axis=0).astype(np.float32)



# revision 23
# speedup vs baseline: 1.3274x; 1.0042x over previous
"""Trainium2 Bass kernel for nn_Discriminator (W_down projection + time-embed
+ W_vt/W_ih projections + 16-step LSTM + linear head).

Strategy: phase A (the dominant 42-GFLOP W_down contraction) is sharded over
the CONTRACTION dim N across the 8 cores: core k holds W_down columns
[k*2500, (k+1)*2500) (2.6 MB fp16 instead of 20.6 MB replicated) and the
matching slice of v for ALL batches. Each core computes partial
vdT [512, 2048] for the full batch; fp16 ReduceScatters (CCE adds on the
SDMA path, compute engines stay free) hand core j the reduced vdT [512, 256]
for its batch block. This turns phase A from DMA-bound (43 MB/core) into
compute-bound (~68 us).

The reduction is split into two collectives by l-half (columns l<8 / l>=8 of
every batch block): phase A computes the l<8 columns first, so RS1 + phases
B1/C1 + LSTM steps 0-7 all overlap RS2. Within each half the psum is retired
per j-half (two 4-bank passes over the k range), so the psum->SBUF copies
and the bounce DMA for target cores 0-3 start at the midpoint of the half's
matmul stream — RS1 issues ~8us earlier and hides fully behind phase A's
half-1 matmuls. Global v columns are laid out h-major (col = lhalf*1024 +
block*128 + (l%8)*16 + b) so each half streams contiguously into a single
40KB/partition SBUF buffer (half 1 re-streams during half 0's matmuls),
which frees room to preload W_ih/W_hh CONCURRENTLY with phase A — phase C
never waits on weight DMA. DMA queues: sync = v/wd stream (single k-tile
chunks up front so the first matmuls start early) then W_ih/W_hh in fine
chunks (so bounce DMAs never queue long behind them), scalar = consts +
collective bounce traffic, gpsimd = the collectives.

Downstream stays batch-parallel (16 batches/core) in T layout
(feature-on-partition). W_hh is fp8e4m3 (lhsT fp8 x rhs fp16 is legal):
~4e-3 total error vs the 2e-2 budget, 4 MB less DMA, and the LSTM's
per-step 256 weight-tile reload gets the faster fp8 fast-weight-load path.
LSTM elementwise runs on DVE with activations on ACT (gpsimd Q7 ucode ops
are not supported by this NEFF path). Per step the gates stream as psum
banks (f,g) and (i,o): the scheduler retires pos0 of both banks first, so f
lands first and o last — fc/ig/c/tanh(c) hide under o's matmuls and the
post-stream tail is only sigmoid(o) + h-mul, retired in 64-col halves so
the next step's first k-tiles start early.
"""

import numpy as np
import ml_dtypes

import concourse.bass as bass
import concourse.bacc as bacc
import concourse.tile as tile
from concourse import mybir
from concourse.bass_utils import run_bass_kernel_spmd

F32 = mybir.dt.float32
F16 = mybir.dt.float16
F8 = mybir.dt.float8e4
NP_F8 = ml_dtypes.float8_e4m3

B, L, N = 128, 16, 20000
HIN, H, HT = 512, 1024, 128
G = 4 * H                     # 4096 gate rows
NCORES = 8
BLOC = B // NCORES            # 16 batches per core
R = BLOC * L                  # 256 cols per batch block (l-major)
RH = R // 2                   # 128 cols per l-half
RHT = RH * NCORES             # 1024 global cols per l-half
RTOT = B * L                  # 2048 global cols
P = 128
NSH = N // NCORES             # 2500 contraction rows per core
KT = 20                       # k-tiles per core (2500 -> 2560 padded)
NPAD = KT * P                 # 2560
DT = HIN // P                 # 4 vd feature tiles
KVT = (HIN + HT) // P         # 5 vt contraction tiles
MC = G // P                   # 32 gate row-tiles
KC = HIN // P                 # 4 xg contraction tiles
KH = H // P                   # 8 lstm contraction tiles
HH = HT // 2                  # 64-col halves of the lstm state

_CACHE = {}


def _build(phases=("A", "B", "C", "L"), null=False, reps=1, cache=True):
    """Build + compile the SPMD Bass module once.

    phases/null/reps are ablation & timing knobs for performance experiments;
    the graded path always builds the full kernel with reps=1.
    """
    key = (tuple(phases), null, reps)
    if cache and key in _CACHE:
        return _CACHE[key]

    nc = bacc.Bacc("TRN2", target_bir_lowering=False, debug=False,
                   num_devices=NCORES)

    d_vT = nc.dram_tensor("vT", [NPAD, RTOT], F16, kind="ExternalInput")
    d_WdT = nc.dram_tensor("WdT", [NPAD, HIN], F16, kind="ExternalInput")
    d_WvtT = nc.dram_tensor("WvtT", [HIN + HT, HIN], F16, kind="ExternalInput")
    d_WihT = nc.dram_tensor("WihT", [HIN, G], F16, kind="ExternalInput")
    d_WhhT = nc.dram_tensor("WhhT8", [H, G], F8, kind="ExternalInput")
    d_t = nc.dram_tensor("t_row", [1, R], F32, kind="ExternalInput")
    d_wt = nc.dram_tensor("wt_row", [1, HT], F32, kind="ExternalInput")
    d_wtb = nc.dram_tensor("wtb_row", [1, HT], F32, kind="ExternalInput")
    d_bias = nc.dram_tensor("bias_g", [P, MC], F32, kind="ExternalInput")
    d_linw = nc.dram_tensor("lin_wT", [P, KH], F16, kind="ExternalInput")
    d_linb = nc.dram_tensor("lin_b_col", [BLOC, 1], F32, kind="ExternalInput")
    d_pred = nc.dram_tensor("pred", [BLOC, 1], F32, kind="ExternalOutput")

    SIG = mybir.ActivationFunctionType.Sigmoid
    TANH = mybir.ActivationFunctionType.Tanh
    IDENT = mybir.ActivationFunctionType.Identity

    with tile.TileContext(nc) as tc:
        with (
            tc.tile_pool(name="const", bufs=1) as const,
            tc.tile_pool(name="ws", bufs=2) as ws,
            tc.tile_pool(name="h16", bufs=2) as h16pool,
            tc.tile_pool(name="dram", bufs=1, space="DRAM") as dram,
        ):
            if null:
                linb_sb = const.tile([BLOC, 1], F32)
                nc.sync.dma_start(out=linb_sb, in_=d_linb[:])
                pred_sb = const.tile([BLOC, 1], F32)
                nc.scalar.copy(out=pred_sb, in_=linb_sb)
                nc.sync.dma_start(out=d_pred[:], in_=pred_sb)
            else:
                _emit(nc, tc, const, ws, h16pool, dram, phases, reps,
                      d_vT, d_WdT, d_WvtT, d_WihT, d_WhhT, d_t,
                      d_wt, d_wtb, d_bias, d_linw, d_linb, d_pred,
                      SIG, TANH, IDENT)

    nc.compile()
    if cache:
        _CACHE[key] = nc
    return nc


def _emit(nc, tc, const, ws, h16pool, dram, phases, reps,
          d_vT, d_WdT, d_WvtT, d_WihT, d_WhhT, d_t, d_wt, d_wtb,
          d_bias, d_linw, d_linb, d_pred, SIG, TANH, IDENT):
    te_sb = const.tile([P, R], F16)            # time embedding, fp16
    vd2_sb = const.tile([P, DT * R], F16)      # reduced vdT for our block
    inpT_sb = const.tile([P, KC * R], F16)     # inputsT
    xgT_sb = const.tile([P, MC * R], F16)      # 16KB/part
    cT = const.tile([P, HT], F32)              # cell state [128, 8*16]
    wvt_sb = const.tile([P, KVT * HIN], F16)   # 5KB/part
    bias_sb = const.tile([P, MC], F32)
    linw_sb = const.tile([P, KH], F16)
    linb_sb = const.tile([BLOC, 1], F32)
    t_sb = const.tile([1, R], F32)
    wt_sb = const.tile([1, HT], F32)
    wtb_sb = const.tile([1, HT], F32)
    ones_sb = const.tile([1, R], F32)

    # collective bounce buffers (DRAM), one pair per l-half. Layout
    # [j, p, d, c]: the (d, c) tail is 1KB-contiguous per partition for fast
    # DMA, and rank j's flat ReduceScatter shard is exactly [p, d, c].
    in_bs = [dram.tile([NCORES, P, DT, RH], F16, name=f"in_b{h}")
             for h in range(2)]
    out_bs = [dram.tile([P, DT, RH], F16, name=f"out_b{h}")
              for h in range(2)]

    wdT3 = d_WdT.rearrange("(a p) d -> p a d", p=P)   # [128, KT, 512]
    vT4 = d_vT.rearrange("(a p) (h q) -> p a h q", p=P, h=2)
    xg3 = xgT_sb.rearrange("p (m r) -> p m r", m=MC)  # [128, 32, 256]
    vd23 = vd2_sb.rearrange("p (d c) -> p d c", d=DT)

    # small constants on the vector DMA queue (keeps sync free for v/wd)
    for k in range(KVT):
        nc.scalar.dma_start(out=wvt_sb[:, k * HIN:(k + 1) * HIN],
                            in_=d_WvtT[k * P:(k + 1) * P, :])
    nc.scalar.dma_start(out=bias_sb, in_=d_bias[:])
    nc.scalar.dma_start(out=linw_sb, in_=d_linw[:])
    nc.scalar.dma_start(out=linb_sb, in_=d_linb[:])
    nc.scalar.dma_start(out=t_sb, in_=d_t[:])
    nc.scalar.dma_start(out=wt_sb, in_=d_wt[:])
    nc.scalar.dma_start(out=wtb_sb, in_=d_wtb[:])
    nc.vector.memset(ones_sb, 1.0)

    for rep in range(reps):
        # ---- LSTM/C weights: preloaded concurrently with phase A; they
        # ride the sync queue BEHIND the phase-A stream so they never
        # starve it (wih lands ~47us, whh ~58us, both before first use)
        wpool_cm = tc.tile_pool(name="wpool", bufs=1)
        wpool = wpool_cm.__enter__()
        whh_sb = wpool.tile([P, KH * G], F8, tag="whh_sb",
                            name="whh_sb")                         # 32KB/part
        wih_sb = wpool.tile([P, KC * G], F16, tag="wih_sb",
                            name="wih_sb")                         # 32KB/part

        # ---- phase A: partial vdT[d, r] += WdT[n,d].T @ vT[n,r], all r ----
        with (
            tc.tile_pool(name="vpool", bufs=1) as vpool,
            tc.tile_pool(name="psa", bufs=1, space="PSUM") as psa,
        ):
            wd_res = vpool.tile([P, KT, HIN], F16, tag="wd_res",
                                name="wd_res")                      # 20KB/part
            # one l-half of v resident at a time (40KB/part); half 1
            # re-streams into the same buffer behind half 0's matmuls
            v_res = vpool.tile([P, KT, RHT], F16, tag="v_res",
                               name="v_res")
            # single k-tile chunks up front so the first matmuls start
            # ~1.5us earlier; pairs after that for DMA efficiency
            ksizes = [1, 1, 1, 1] + [2] * ((KT - 4) // 2)
            for half in range(2 if "A" in phases else 0):
                k0 = 0
                for sz in ksizes:
                    if half == 0:
                        nc.sync.dma_start(out=wd_res[:, k0:k0 + sz, :],
                                          in_=wdT3[:, k0:k0 + sz, :])
                    nc.sync.dma_start(out=v_res[:, k0:k0 + sz, :],
                                      in_=vT4[:, k0:k0 + sz, half, :])
                    k0 += sz
                if half == 1:
                    # weights queue behind the full v stream; fine chunks so
                    # the half-0 bounce DMA never waits long behind them
                    for k in range(KC):
                        for c in range(2):
                            nc.sync.dma_start(
                                out=wih_sb[:, k * G + c * (G // 2):
                                           k * G + (c + 1) * (G // 2)],
                                in_=d_WihT[k * P:(k + 1) * P,
                                           c * (G // 2):(c + 1) * (G // 2)])
                    for k in range(KH):
                        nc.sync.dma_start(out=whh_sb[:, k * G:(k + 1) * G],
                                          in_=d_WhhT[k * P:(k + 1) * P, :])
                # psum retired per j-half: copies + bounce of target cores
                # 0-3 start at the midpoint of this half's matmul stream
                psA = [[psa.tile([P, RHT // 2], F32, tag=f"psA{d}{jh}",
                                 name=f"psA{d}{jh}_{half}") for d in range(DT)]
                       for jh in range(2)]
                # stage fp16 partials j-major so the bounce DMA collapses
                # to 3 dims: vdp cols = j*(DT*128) + d*128 + c
                vdp = vpool.tile([P, DT * RHT], F16, tag="vdp",
                                 name=f"vdp_{half}", bufs=2)
                vdp4 = vdp.rearrange("p (j d c) -> p j d c", j=NCORES, d=DT)
                vdp3 = vdp.rearrange("p (j dc) -> p j dc", j=NCORES)
                in_b3 = in_bs[half].rearrange("j p d c -> p j (d c)")
                for jh in range(2):
                    for k in range(KT):
                        for d in range(DT):
                            nc.tensor.matmul(
                                psA[jh][d],
                                lhsT=wd_res[:, k, d * P:(d + 1) * P],
                                rhs=v_res[:, k, jh * 512:(jh + 1) * 512],
                                start=(k == 0), stop=(k == KT - 1))
                    for d in range(DT):
                        psA3 = psA[jh][d].rearrange("p (j c) -> p j c",
                                                    j=NCORES // 2)
                        dst = vdp4[:, jh * 4:(jh + 1) * 4, d, :]
                        if d % 2 == 1:
                            nc.scalar.copy(out=dst, in_=psA3)
                        else:
                            nc.vector.tensor_copy(out=dst, in_=psA3)
                    nc.scalar.dma_start(
                        out=in_b3[:, jh * 4:(jh + 1) * 4, :],
                        in_=vdp3[:, jh * 4:(jh + 1) * 4, :])
                nc.gpsimd.collective_compute(
                    "ReduceScatter", mybir.AluOpType.add,
                    replica_groups=[list(range(NCORES))],
                    ins=[in_bs[half].opt()], outs=[out_bs[half].opt()])
                # readback: our block's columns for this l-half
                nc.scalar.dma_start(
                    out=vd23[:, :, half * RH:(half + 1) * RH],
                    in_=out_bs[half].rearrange("p d c -> p d c"))

        with tc.tile_pool(name="psmm", bufs=1, space="PSUM") as psmm:
            # te: wt[j] * t[r] + wtb[j]  (rank-1 matmuls, K=1)
            psTE = psmm.tile([P, R], F32, tag="psTE", name="psTE")
            nc.tensor.matmul(psTE, lhsT=wt_sb, rhs=t_sb, start=True,
                             stop=False)
            nc.tensor.matmul(psTE, lhsT=wtb_sb, rhs=ones_sb, start=False,
                             stop=True)
            nc.vector.tensor_copy(out=te_sb, in_=psTE)

            h_prev = None
            for lh in range(2):
                c0 = lh * RH
                # B/C of the first l-half gate the LSTM's serial chain:
                # raise their priority so they preempt phase A's half-1
                # matmul stream the moment the RS readback lands
                prio_cm = tc.high_priority() if lh == 0 else None
                if prio_cm is not None:
                    prio_cm.__enter__()
                # ---- phase B (this l-half): inputsT over [vd2; te] ----
                for m in range(KC if "B" in phases else 0):
                    psB = psmm.tile([P, RH], F32, tag="psBC",
                                    name=f"psB{m}_{lh}", bufs=3)
                    for k in range(KVT):
                        rhs = (vd23[:, k, c0:c0 + RH] if k < KC
                               else te_sb[:, c0:c0 + RH])
                        nc.tensor.matmul(
                            psB,
                            lhsT=wvt_sb[:, k * HIN + m * P:
                                        k * HIN + (m + 1) * P],
                            rhs=rhs, start=(k == 0), stop=(k == KVT - 1))
                    nc.vector.tensor_copy(
                        out=inpT_sb[:, m * R + c0:m * R + c0 + RH], in_=psB)

                # ---- phase C (this l-half): xgT = WihT.T @ inputsT + b ----
                for m in range(MC if "C" in phases else 0):
                    psC = psmm.tile([P, RH], F32, tag="psBC",
                                    name=f"psC{m}_{lh}", bufs=3)
                    for k in range(KC):
                        nc.tensor.matmul(
                            psC,
                            lhsT=wih_sb[:, k * G + m * P:k * G + (m + 1) * P],
                            rhs=inpT_sb[:, k * R + c0:k * R + c0 + RH],
                            start=(k == 0), stop=(k == KC - 1))
                    if m % 2 == 0:
                        nc.scalar.activation(
                            out=xgT_sb[:, m * R + c0:m * R + c0 + RH],
                            in_=psC, func=IDENT,
                            bias=bias_sb[:, m:m + 1], scale=1.0)
                    else:
                        nc.vector.tensor_scalar_add(
                            xgT_sb[:, m * R + c0:m * R + c0 + RH],
                            psC, bias_sb[:, m:m + 1])
                if prio_cm is not None:
                    prio_cm.__exit__(None, None, None)

                # ---- LSTM steps of this l-half ----
                for t in range(lh * 8, (lh + 1) * 8 if "L" in phases else 0):
                    acts = [None] * 4
                    if t == 0:
                        # f-gate is dead at t=0 (c0 = 0); gates = xg directly
                        for g in (0, 2, 3):
                            a = ws.tile([P, HT], F32, tag=f"act{g}",
                                        name=f"act{g}_{t}")
                            a3 = a.rearrange("p (m j) -> p m j", m=KH)
                            nc.scalar.activation(
                                out=a3, in_=xg3[:, g * KH:(g + 1) * KH,
                                                t * BLOC:(t + 1) * BLOC],
                                func=(TANH if g == 2 else SIG))
                            acts[g] = a
                        i_a, g_a, o_a = acts[0], acts[2], acts[3]
                        nc.vector.tensor_mul(cT, i_a, g_a)
                        h_new = [None, None]
                        for hh in range(2):
                            sl = slice(hh * HH, (hh + 1) * HH)
                            tc_t = ws.tile([P, HH], F32, tag=f"tanhc{hh}",
                                           name=f"tanhc{hh}_{t}")
                            nc.scalar.activation(out=tc_t, in_=cT[:, sl],
                                                 func=TANH)
                            hn = h16pool.tile([P, HH], F16, tag=f"h16{hh}",
                                              name=f"h16{hh}_{t}")
                            nc.vector.tensor_mul(hn, o_a[:, sl], tc_t)
                            h_new[hh] = hn
                        h_prev = h_new
                        continue
                    # psum banks (f,g) and (i,o): the scheduler retires pos0
                    # of both banks first, then pos1 — so f lands first and o
                    # last under either pattern. The c chain (fc, ig, c,
                    # tanh) hides under o's matmuls; the tail is only o's
                    # act halves + h muls.
                    pshs = [psmm.tile([P, 2 * HT], F32, tag=f"psL{b}",
                                      name=f"psL{b}_{t}", bufs=2)
                            for b in range(2)]
                    order = [1, 2, 0, 3]               # f, g, i, o
                    for gi, g in enumerate(order):
                        b, pos = divmod(gi, 2)
                        ph = pshs[b]
                        for mt in range(KH):
                            m = g * KH + mt
                            mm = pos * KH + mt
                            sl = slice(mm * BLOC, (mm + 1) * BLOC)
                            for k in range(KH):
                                hp = h_prev[k // 4]
                                nc.tensor.matmul(
                                    ph[:, sl],
                                    lhsT=whh_sb[:, k * G + m * P:
                                                k * G + (m + 1) * P],
                                    rhs=hp[:, (k % 4) * BLOC:
                                           (k % 4 + 1) * BLOC],
                                    start=(k == 0), stop=(k == KH - 1))
                        if g == 3:                 # o: pre-add in halves below
                            ph_o, pos_o = ph, pos
                            continue
                        ph3 = ph.rearrange("p (m j) -> p m j", m=MC // 2)
                        pre = ws.tile([P, HT], F32, tag=f"pre{g}",
                                      name=f"pre{g}_{t}")
                        pre3 = pre.rearrange("p (m j) -> p m j", m=KH)
                        nc.vector.tensor_add(
                            pre3,
                            ph3[:, pos * KH:(pos + 1) * KH, :],
                            xg3[:, g * KH:(g + 1) * KH,
                                t * BLOC:(t + 1) * BLOC])
                        a = ws.tile([P, HT], F32, tag=f"act{g}",
                                    name=f"act{g}_{t}")
                        nc.scalar.activation(out=a, in_=pre,
                                             func=(TANH if g == 2 else SIG))
                        acts[g] = a
                        if g == 1:                 # fc as soon as f retires
                            fc = ws.tile([P, HT], F32, tag="fc",
                                         name=f"fc_{t}")
                            nc.vector.tensor_mul(fc, a, cT)
                        elif g == 0:               # c update under o's MMs
                            ig = ws.tile([P, HT], F32, tag="ig",
                                         name=f"ig_{t}")
                            nc.vector.tensor_mul(ig, a, acts[2])
                            nc.vector.tensor_add(cT, ig, fc)
                    # tail, in 64-col halves: o's pre-add for half 0 starts
                    # after o's first 4 m-tiles stop (mid-stream), so after
                    # o's last matmul only sigmoid(o1) + h muls remain; next
                    # step's first k-tiles start once h half 0 lands
                    ph_o3 = ph_o.rearrange("p (m j) -> p m j", m=MC // 2)
                    h_new = [None, None]
                    for hh in range(2):
                        sl = slice(hh * HH, (hh + 1) * HH)
                        m0 = hh * (KH // 2)
                        tc_t = ws.tile([P, HH], F32, tag=f"tanhc{hh}",
                                       name=f"tanhc{hh}_{t}")
                        nc.scalar.activation(out=tc_t, in_=cT[:, sl],
                                             func=TANH)
                        pre_o = ws.tile([P, HH], F32, tag=f"preo{hh}",
                                        name=f"preo{hh}_{t}")
                        pre_o3 = pre_o.rearrange("p (m j) -> p m j",
                                                 m=KH // 2)
                        nc.vector.tensor_add(
                            pre_o3,
                            ph_o3[:, pos_o * KH + m0:pos_o * KH + m0 + 4, :],
                            xg3[:, 3 * KH + m0:3 * KH + m0 + 4,
                                t * BLOC:(t + 1) * BLOC])
                        o_h = ws.tile([P, HH], F32, tag=f"oact{hh}",
                                      name=f"oact{hh}_{t}")
                        nc.scalar.activation(out=o_h, in_=pre_o, func=SIG)
                        hn = h16pool.tile([P, HH], F16, tag=f"h16{hh}",
                                          name=f"h16{hh}_{t}")
                        nc.vector.tensor_mul(hn, o_h, tc_t)
                        h_new[hh] = hn
                    h_prev = h_new

            # ---- head: pred = hT.T @ lin_wT + lin_b ----
            if h_prev is None:
                h_prev = [h16pool.tile([P, HH], F16, tag=f"h16{hh}",
                                       name=f"h16_d{hh}") for hh in range(2)]
                for hh in range(2):
                    nc.vector.memset(h_prev[hh], 0.0)
            psP = psmm.tile([BLOC, 1], F32, tag="psTE", name="psP")
            for k in range(KH):
                hp = h_prev[k // 4]
                nc.tensor.matmul(psP,
                                 lhsT=hp[:, (k % 4) * BLOC:
                                        (k % 4 + 1) * BLOC],
                                 rhs=linw_sb[:, k:k + 1],
                                 start=(k == 0), stop=(k == KH - 1))
            pred_sb = const.tile([BLOC, 1], F32, name="pred_sb",
                                 tag="pred_sb")
            nc.scalar.activation(out=pred_sb, in_=psP, func=IDENT,
                                 bias=linb_sb, scale=1.0)
            nc.sync.dma_start(out=d_pred[:], in_=pred_sb)
        wpool_cm.__exit__(None, None, None)


def _prep_in_maps(v, t, W_down, Wt_up_w, Wt_up_b, W_vt, W_ih, W_hh,
                  b_ih, b_hh, lin_w, lin_b):
    """Host-side shard/layout/dtype prep. Layout + cast only, no math."""
    WvtT = np.ascontiguousarray(W_vt.T).astype(np.float16)
    WihT = np.ascontiguousarray(W_ih.T).astype(np.float16)
    WhhT8 = np.ascontiguousarray(W_hh.T).astype(NP_F8)
    wt_row = np.ascontiguousarray(Wt_up_w.reshape(1, HT)).astype(np.float32)
    wtb_row = np.ascontiguousarray(Wt_up_b.reshape(1, HT)).astype(np.float32)
    bias_g = np.ascontiguousarray(
        (b_ih + b_hh).astype(np.float32).reshape(MC, P).T)
    lin_wT = np.ascontiguousarray(
        lin_w.reshape(KH, P).T).astype(np.float16)
    lin_b_col = np.full((BLOC, 1), np.float32(lin_b[0]), np.float32)

    # global columns h-major: col = (l//8)*1024 + bblock*128 + (l%8)*16 + b
    vg = np.ascontiguousarray(
        v.reshape(NCORES, BLOC, 2, L // 2, N)
        .transpose(4, 2, 0, 3, 1).reshape(N, RTOT))

    shared = dict(WvtT=WvtT, WihT=WihT, WhhT8=WhhT8, wt_row=wt_row,
                  wtb_row=wtb_row, bias_g=bias_g, lin_wT=lin_wT,
                  lin_b_col=lin_b_col)
    in_maps = []
    for c in range(NCORES):
        n0 = c * NSH
        vT = np.zeros((NPAD, RTOT), np.float16)
        vT[:NSH] = vg[n0:n0 + NSH].astype(np.float16)
        WdT = np.zeros((NPAD, HIN), np.float16)
        WdT[:NSH] = W_down[:, n0:n0 + NSH].T.astype(np.float16)
        b0 = c * BLOC
        t_row = np.ascontiguousarray(
            t[b0:b0 + BLOC].T.reshape(1, R)).astype(np.float32)
        in_maps.append(dict(vT=vT, WdT=WdT, t_row=t_row, **shared))
    return in_maps


def kernel(**inputs):
    nc = _build()
    in_maps = _prep_in_maps(**inputs)
    res = run_bass_kernel_spmd(nc, in_maps, core_ids=list(range(NCORES)))
    return np.concatenate([res.results[c]["pred"] for c in range(NCORES)],
                          axis=0).astype(np.float32)

